# revision 13
# baseline (speedup 1.0000x reference)
"""Multi-head attention Bass kernel for Trainium2 (8 NeuronCores).

Problem: B=8, T=2048, C=256, H=8, D=32 MHA (dense, full softmax over T).
Sharding: data-parallel over batch -- core b computes batch b end-to-end,
no collectives.  Weights are replicated; per-core x slice is [T, C].

Per-core dataflow (v3):
  1. x is pre-cast to bf16 on the host; xT [C,T] is produced by TWO DMA
     xbar-transpose instructions reading straight from DRAM (no staging,
     no casts, no PE transposes).
  2. qT/kT [D,T] packed 4 heads per [128,T] tile via M=128 matmuls
     (8 MMs per group; group-0 PSUM evacuations on ScalarE so the exp
     stream starts ~8us in); then v [T,D] per head with an appended
     ones-column (v_ext [T,33], bf16).  wq pre-scaled by 1/sqrt(C)=1/16
     host-side.
  3. Scores TRANSPOSED: weiT[s,t] = k[s]*q[t] per head (bf16 matmuls, fp32
     PSUM); ScalarE exp() PSUM->SBUF(bf16): 256 instrs of [128,1024] -- the
     critical path.  Everything else is structured to keep ScalarE fed.
  4. AV: outT_ext[33,t] = v_ext.T @ expw accumulated over s-chunks in PSUM
     (2 heads per tile at partitions 0/64; rows 32/96 are the softmax
     denominators via the ones column).
  5. Normalize with minimal PSUM dwell: two DVE copies evacuate the
     accumulator right after the s-loop (banks free for the next pair
     ~5us after it ends), then off the critical path: reciprocal of the
     denominator rows, broadcast to 32 partitions via a DRAM-roundtrip
     DMA with a zero-stride source AP, and a DVE multiply into nout
     (bf16, zero-padded rows for the projection).  The last pair instead
     broadcasts via K=1 PE matmuls into the idle scores banks (shorter
     chain; no next pair to protect).
  6. Projection: res = sum_pairs noutT_p.T @ wproj_p + bias (bf16 matmuls,
     wproj zero-padded host-side to the pair layout); output DMAs on the
     (idle) ScalarE HWDGE queue.
"""

import numpy as np
import ml_dtypes
from contextlib import ExitStack

import concourse.bass as bass
import concourse.bacc as bacc
import concourse.mybir as mybir
import concourse.tile as tile
from concourse.bass_utils import run_bass_kernel_spmd

B, T, C, H, D = 8, 2048, 256, 8, 32
P = 128
NT = T // P  # 16 chunks of 128 along t / s
F32 = mybir.dt.float32
BF16 = mybir.dt.bfloat16
EXP = mybir.ActivationFunctionType.Exp
N_CORES = 8
E = D + 1  # 33: v columns + ones column


def _body(nc, tc, ctx, x_d, wq_d, wk_d, wv_d, wp_d, bias_d, out_d):
    const = ctx.enter_context(tc.tile_pool(name="const", bufs=1))
    big = ctx.enter_context(tc.tile_pool(name="big", bufs=1))

    wq_sb = const.tile([P, 2, C], BF16)
    wk_sb = const.tile([P, 2, C], BF16)
    wv_sb = const.tile([P, 2, C], BF16)
    wp_sb = const.tile([P, 4, C], BF16)
    bias_sb = const.tile([P, C], F32)
    ones_sb = const.tile([P, D], BF16)
    warm = const.tile([P, 1], F32)

    xT = [big.tile([P, T], BF16, name=f"xT{i}") for i in range(2)]
    qT = [big.tile([P, T], BF16, name=f"qT{i}") for i in range(2)]
    kT = [big.tile([P, T], BF16, name=f"kT{i}") for i in range(2)]
    v_sb = big.tile([P, NT, E * H], BF16)
    nout = [big.tile([P, T], BF16, name=f"nout{i}") for i in range(4)]

    # ---- Phase 1: xT via DRAM xbar transpose FIRST (Tile serializes DMA
    # xbar-mode transitions, so all plain DMAs go after); bf16 weights are
    # DMA'd straight into their SBUF tiles (host pre-casts) ---------------
    for cc in range(2):
        nc.scalar.dma_start_transpose(
            out=xT[cc], in_=x_d[:, cc * P:(cc + 1) * P])
    for w_sb, w_d, nk in ((wq_sb, wq_d, 2), (wk_sb, wk_d, 2),
                          (wv_sb, wv_d, 2), (wp_sb, wp_d, 4)):
        if nk == 2:
            nc.scalar.dma_start(
                out=w_sb, in_=w_d.rearrange("(k p) c -> p k c", p=P))
        else:
            nc.scalar.dma_start(out=w_sb, in_=w_d.rearrange("q p c -> p q c"))
    nc.scalar.dma_start(out=bias_sb, in_=bias_d)
    nc.gpsimd.memset(v_sb, 1.0)  # ones cols survive; v overwrites the rest
    for t_ in nout:  # rows 32-63 / 96-127 must be 0.0 for the projection
        nc.gpsimd.memset(t_, 0.0)
    nc.vector.memset(ones_sb, 1.0)
    nc.scalar.activation(out=warm, in_=ones_sb[:, 0:1], func=EXP)

    # ---- Phase 2: v (warms the PE), then qT / kT (M=128 matmuls) --------
    with tc.tile_pool(name="pv", bufs=2, space="PSUM") as pv:
        for n in range(NT):
            vp = pv.tile([P, C], F32, tag="vp", name="vp")
            for cc in range(2):
                nc.tensor.matmul(
                    vp,
                    lhsT=xT[cc][:, n * P:(n + 1) * P],
                    rhs=wv_sb[:, cc, :],
                    start=(cc == 0), stop=(cc == 1))
            nc.vector.tensor_copy(
                v_sb[:, n].rearrange("p (h e) -> p h e", e=E)[:, :, 0:D],
                vp.rearrange("p (h d) -> p h d", d=D))
    with tc.tile_pool(name="pq", bufs=2, space="PSUM") as pq:
        for g in range(2):
            for w_sb, dest in ((wq_sb, qT), (wk_sb, kT)):
                qp = pq.tile([P, T], F32, tag="qp", name="qp")
                for ts in range(4):
                    for cc in range(2):
                        nc.tensor.matmul(
                            qp[:, 512 * ts:512 * (ts + 1)],
                            lhsT=w_sb[:, cc, P * g:P * (g + 1)],
                            rhs=xT[cc][:, 512 * ts:512 * (ts + 1)],
                            start=(cc == 0), stop=(cc == 1))
                if g == 0:  # ScalarE is idle pre-attention: fastest start
                    nc.scalar.copy(dest[g], qp)
                else:
                    nc.vector.tensor_copy(dest[g], qp)

    # ---- Phase 3: attention, head pairs ---------------------------------
    with tc.tile_pool(name="scp", bufs=2, space="PSUM") as scp, \
         tc.tile_pool(name="avp", bufs=1, space="PSUM") as avp, \
         tc.tile_pool(name="expp", bufs=6) as expp, \
         tc.tile_pool(name="nrm", bufs=2) as nrm, \
         tc.tile_pool(name="dsc", bufs=2, space="DRAM") as dscp:
        for pair in range(4):
            g = pair // 2
            hA, hB = 2 * pair, 2 * pair + 1
            last = pair == 3
            av = avp.tile([P, T], F32, tag="av", name="av")
            for s in range(NT):
                exs = []
                for h in (hA, hB):
                    j = h % 4
                    for half in range(2):
                        sc = scp.tile([P, 1024], F32, tag="sc", name="sc")
                        for ts in range(2):
                            tofs = 1024 * half + 512 * ts
                            nc.tensor.matmul(
                                sc[:, 512 * ts:512 * (ts + 1)],
                                lhsT=kT[g][D * j:D * (j + 1), P * s:P * (s + 1)],
                                rhs=qT[g][D * j:D * (j + 1), tofs:tofs + 512],
                                start=True, stop=True,
                                tile_position=(D * j, 0))
                        ex = expp.tile([P, 1024], BF16, tag="ex", name="ex")
                        nc.scalar.activation(out=ex, in_=sc, func=EXP)
                        exs.append((h, half, ex))
                for h, half, ex in exs:
                    col = 0 if h == hA else 64
                    for ts in range(2):
                        tofs = 1024 * half + 512 * ts
                        # A and B share banks at different partition ranges;
                        # per-element has_written makes that safe on HW, but
                        # the sim's bank-granular group tracker would flag it.
                        nc.tensor.matmul(
                            av[col:col + E, tofs:tofs + 512],
                            lhsT=v_sb[:, s, E * h:E * h + E],
                            rhs=ex[:, 512 * ts:512 * (ts + 1)],
                            start=(s == 0), stop=(s == NT - 1),
                            tile_position=(0, col),
                            skip_group_check=True)
            avf = nrm.tile([P, T], F32, tag="avf", name="avf")
            rec = nrm.tile([P, T], F32, tag="rec", name="rec")
            if not last:
                # Evacuate PSUM promptly (two DVE copies; banks free ~5us
                # after the pair's last AV matmul), then off the critical
                # path: reciprocal + DRAM-roundtrip broadcast + multiply.
                for base in (0, 64):
                    nc.vector.tensor_copy(avf[base:base + E, :],
                                          av[base:base + E, :])
                dsc = dscp.tile([2, T], F32, tag="dsc", name="dsc")
                bc = nrm.tile([P, T], F32, tag="bc", name="bc")
                for i, row in enumerate((D, 64 + D)):
                    # reciprocal_approx_fast (custom DVE) gives garbage on HW
                    # via this path; the iterative-divide reciprocal is exact.
                    nc.vector.reciprocal(
                        out=rec[row:row + 1, :], in_=avf[row:row + 1, :])
                    nc.sync.dma_start(out=dsc[i:i + 1, :],
                                      in_=rec[row:row + 1, :])
                for i, base in enumerate((0, 64)):
                    nc.sync.dma_start(out=bc[base:base + D, :],
                                      in_=dsc[i, :].partition_broadcast(D))
                    nc.vector.tensor_mul(
                        nout[pair][base:base + D, :],
                        avf[base:base + D, :],
                        bc[base:base + D, :])
            else:
                # Last pair: no next-pair PSUM pressure.  Shortest chain:
                # DVE reciprocal straight from PSUM while ScalarE (idle now)
                # evacuates the value rows; K=1 PE matmuls broadcast the
                # reciprocal into the idle scores banks.
                rec2 = nrm.tile([P, T], BF16, tag="rec2", name="rec2")
                for i, row in enumerate((D, 64 + D)):
                    nc.vector.reciprocal(
                        out=rec[row:row + 1, :], in_=av[row:row + 1, :])
                    nc.vector.tensor_copy(rec2[row:row + 1, :],
                                          rec[row:row + 1, :])
                for base in (0, 64):
                    nc.scalar.copy(avf[base:base + D, :], av[base:base + D, :])
                for half in range(2):
                    bcp = scp.tile([P, 1024], F32, tag="sc", name="bcp")
                    for colofs, row in ((0, D), (64, 64 + D)):
                        for ts in range(2):
                            tofs = 1024 * half + 512 * ts
                            nc.tensor.matmul(
                                bcp[colofs:colofs + D, 512 * ts:512 * (ts + 1)],
                                lhsT=ones_sb[row:row + 1, :],
                                rhs=rec2[row:row + 1, tofs:tofs + 512],
                                start=True, stop=True,
                                tile_position=((row // 32) * 32, colofs),
                                skip_group_check=True)
                    for base in (0, 64):
                        tofs = 1024 * half
                        nc.vector.tensor_mul(
                            nout[pair][base:base + D, tofs:tofs + 1024],
                            avf[base:base + D, tofs:tofs + 1024],
                            bcp[base:base + D, :])

    # ---- Phase 4: output projection + bias ------------------------------
    with tc.tile_pool(name="prp", bufs=2, space="PSUM") as prp, \
         tc.tile_pool(name="resp", bufs=1) as resp:
        resbig = resp.tile([P, NT, C], F32, name="resbig")
        out_r = out_d.rearrange("(d m p) c -> d p m c", p=P, m=4)
        for n in range(NT):
            rp = prp.tile([P, C], F32, tag="rp", name="rp")
            for q in range(4):
                nc.tensor.matmul(
                    rp,
                    lhsT=nout[q][:, P * n:P * (n + 1)],
                    rhs=wp_sb[:, q, :],
                    start=(q == 0), stop=(q == 3))
            nc.vector.tensor_add(resbig[:, n, :], rp, bias_sb)
            if n % 4 == 3:  # batched output DMA per 4 chunks
                nc.scalar.dma_start(out=out_r[n // 4],
                                    in_=resbig[:, n - 3:n + 1, :])


def declare_io(nc):
    """Declare the kernel's DRAM IO tensors (shared with test harnesses)."""
    x_d = nc.dram_tensor("x", [T, C], BF16, kind="ExternalInput")
    wq_d = nc.dram_tensor("wq", [C, C], BF16, kind="ExternalInput")
    wk_d = nc.dram_tensor("wk", [C, C], BF16, kind="ExternalInput")
    wv_d = nc.dram_tensor("wv", [C, C], BF16, kind="ExternalInput")
    wp_d = nc.dram_tensor("wp", [4, P, C], BF16, kind="ExternalInput")
    bias_d = nc.dram_tensor("bias", [P, C], F32, kind="ExternalInput")
    out_d = nc.dram_tensor("out", [T, C], F32, kind="ExternalOutput")
    return x_d, wq_d, wk_d, wv_d, wp_d, bias_d, out_d


def build_nc():
    nc = bacc.Bacc("TRN2", debug=False, num_devices=N_CORES)
    x_d, wq_d, wk_d, wv_d, wp_d, bias_d, out_d = declare_io(nc)
    with tile.TileContext(nc) as tc:
        with ExitStack() as ctx:
            _body(nc, tc, ctx, x_d.ap(), wq_d.ap(), wk_d.ap(), wv_d.ap(),
                  wp_d.ap(), bias_d.ap(), out_d.ap())
    nc.compile()
    return nc


def prep_inputs(x, wq, wk, wv, wproj, bproj):
    """Host-side reformatting of the full inputs into per-core input maps."""
    f = np.float32
    bf = ml_dtypes.bfloat16
    # [H,C,D] -> [C, H*D]; wq additionally pre-scaled by 1/sqrt(C) (exact).
    wq2 = np.ascontiguousarray(
        np.transpose(np.asarray(wq, f), (1, 0, 2)).reshape(C, H * D)
        * f(1.0 / 16.0)).astype(bf)
    wk2 = np.ascontiguousarray(
        np.transpose(np.asarray(wk, f), (1, 0, 2)).reshape(C, H * D)).astype(bf)
    wv2 = np.ascontiguousarray(
        np.transpose(np.asarray(wv, f), (1, 0, 2)).reshape(C, H * D)).astype(bf)
    # wproj [H*D, C] -> 4 pair-chunks padded to 128 rows:
    # rows 0-31 <- head 2p, rows 64-95 <- head 2p+1, rest zero.
    wp4 = np.zeros((4, P, C), f)
    wproj = np.asarray(wproj, f)
    for p in range(4):
        wp4[p, 0:D] = wproj[64 * p: 64 * p + D]
        wp4[p, 64:64 + D] = wproj[64 * p + D: 64 * p + 2 * D]
    wp4 = wp4.astype(bf)
    bias128 = np.ascontiguousarray(
        np.broadcast_to(np.asarray(bproj, f), (P, C)))
    x_bf = np.asarray(x, f).astype(ml_dtypes.bfloat16)
    in_maps = []
    for b in range(N_CORES):
        in_maps.append({
            "x": np.ascontiguousarray(x_bf[b]),
            "wq": wq2, "wk": wk2, "wv": wv2,
            "wp": wp4, "bias": bias128,
        })
    return in_maps


def kernel(x, wq, wk, wv, wproj, bproj, _nc=None):
    in_maps = prep_inputs(x, wq, wk, wv, wproj, bproj)
    nc = _nc if _nc is not None else build_nc()
    res = run_bass_kernel_spmd(nc, in_maps, list(range(N_CORES)))
    return np.stack([r["out"] for r in res.results], axis=0)


# revision 15
# speedup vs baseline: 8.2778x; 8.2778x over previous
"""Multi-head attention Bass kernel for Trainium2 (8 NeuronCores).

Problem: B=8, T=2048, C=256, H=8, D=32 MHA (dense, full softmax over T).
Sharding: data-parallel over batch -- core b computes batch b end-to-end,
no collectives.  Weights are replicated; per-core x slice is [T, C].

Per-core dataflow (v3):
  1. x is pre-cast to bf16 on the host; xT [C,T] is produced by TWO DMA
     xbar-transpose instructions reading straight from DRAM (no staging,
     no casts, no PE transposes).
  2. qT/kT [D,T] packed 4 heads per [128,T] tile via M=128 matmuls
     (8 MMs per group; group-0 PSUM evacuations on ScalarE so the exp
     stream starts ~8us in); then v [T,D] per head with an appended
     ones-column (v_ext [T,33], bf16).  wq pre-scaled by 1/sqrt(C)=1/16
     host-side.
  3. Scores TRANSPOSED: weiT[s,t] = k[s]*q[t] per head (bf16 matmuls, fp32
     PSUM); ScalarE exp() PSUM->SBUF(bf16): 256 instrs of [128,1024] -- the
     critical path.  Everything else is structured to keep ScalarE fed.
  4. AV: outT_ext[33,t] = v_ext.T @ expw accumulated over s-chunks in PSUM
     (2 heads per tile at partitions 0/64; rows 32/96 are the softmax
     denominators via the ones column).
  5. Normalize with minimal PSUM dwell: two DVE copies evacuate the
     accumulator right after the s-loop (banks free for the next pair
     ~5us after it ends), then off the critical path: reciprocal of the
     denominator rows, broadcast to 32 partitions via a DRAM-roundtrip
     DMA with a zero-stride source AP, and a DVE multiply into nout
     (bf16, zero-padded rows for the projection).  The last pair instead
     broadcasts via K=1 PE matmuls into the idle scores banks (shorter
     chain; no next pair to protect).
  6. Projection: res = sum_pairs noutT_p.T @ wproj_p + bias (bf16 matmuls,
     wproj zero-padded host-side to the pair layout); output DMAs on the
     (idle) ScalarE HWDGE queue.
"""

import numpy as np
import ml_dtypes
from contextlib import ExitStack

import concourse.bass as bass
import concourse.bacc as bacc
import concourse.mybir as mybir
import concourse.tile as tile
from concourse.bass_utils import run_bass_kernel_spmd

B, T, C, H, D = 8, 2048, 256, 8, 32
P = 128
NT = T // P  # 16 chunks of 128 along t / s
F32 = mybir.dt.float32
BF16 = mybir.dt.bfloat16
EXP = mybir.ActivationFunctionType.Exp
N_CORES = 8
E = D + 1  # 33: v columns + ones column


def _body(nc, tc, ctx, x_d, wq_d, wk_d, wv_d, wp_d, bias_d, out_d):
    const = ctx.enter_context(tc.tile_pool(name="const", bufs=1))
    big = ctx.enter_context(tc.tile_pool(name="big", bufs=1))

    wq_sb = const.tile([P, 2, C], BF16)
    wk_sb = const.tile([P, 2, C], BF16)
    wv_sb = const.tile([P, 2, C], BF16)
    wp_sb = const.tile([P, 4, C], BF16)
    bias_sb = const.tile([P, C], F32)
    ones_sb = const.tile([P, D], BF16)
    warm = const.tile([P, 1], F32)

    xT = [big.tile([P, T], BF16, name=f"xT{i}") for i in range(2)]
    qT = [big.tile([P, T], BF16, name=f"qT{i}") for i in range(2)]
    kT = [big.tile([P, T], BF16, name=f"kT{i}") for i in range(2)]
    v_sb = big.tile([P, NT, E * H], BF16)
    nout = [big.tile([P, T], BF16, name=f"nout{i}") for i in range(4)]

    # ---- Phase 1: xT via DRAM xbar transpose FIRST (Tile serializes DMA
    # xbar-mode transitions, so all plain DMAs go after); bf16 weights are
    # DMA'd straight into their SBUF tiles (host pre-casts) ---------------
    for cc, eng in ((0, nc.scalar), (1, nc.sync)):
        eng.dma_start_transpose(
            out=xT[cc], in_=x_d[:, cc * P:(cc + 1) * P])
    for w_sb, w_d, nk in ((wq_sb, wq_d, 2), (wk_sb, wk_d, 2),
                          (wv_sb, wv_d, 2), (wp_sb, wp_d, 4)):
        if nk == 2:
            nc.scalar.dma_start(
                out=w_sb, in_=w_d.rearrange("(k p) c -> p k c", p=P))
        else:
            nc.scalar.dma_start(out=w_sb, in_=w_d.rearrange("q p c -> p q c"))
    nc.scalar.dma_start(out=bias_sb, in_=bias_d)
    nc.gpsimd.memset(v_sb, 1.0)  # ones cols survive; v overwrites the rest
    for t_ in nout:  # rows 32-63 / 96-127 must be 0.0 for the projection
        nc.gpsimd.memset(t_, 0.0)
    nc.vector.memset(ones_sb, 1.0)
    nc.scalar.activation(out=warm, in_=ones_sb[:, 0:1], func=EXP)

    # ---- Phase 2: v (warms the PE), then qT / kT (M=128 matmuls) --------
    with tc.tile_pool(name="pv", bufs=2, space="PSUM") as pv:
        for n in range(NT):
            vp = pv.tile([P, C], F32, tag="vp", name="vp")
            for cc in range(2):
                nc.tensor.matmul(
                    vp,
                    lhsT=xT[cc][:, n * P:(n + 1) * P],
                    rhs=wv_sb[:, cc, :],
                    start=(cc == 0), stop=(cc == 1))
            nc.vector.tensor_copy(
                v_sb[:, n].rearrange("p (h e) -> p h e", e=E)[:, :, 0:D],
                vp.rearrange("p (h d) -> p h d", d=D))
    with tc.tile_pool(name="pq", bufs=2, space="PSUM") as pq:
        for g in range(2):
            for w_sb, dest in ((wq_sb, qT), (wk_sb, kT)):
                qp = pq.tile([P, T], F32, tag="qp", name="qp")
                for ts in range(4):
                    for cc in range(2):
                        nc.tensor.matmul(
                            qp[:, 512 * ts:512 * (ts + 1)],
                            lhsT=w_sb[:, cc, P * g:P * (g + 1)],
                            rhs=xT[cc][:, 512 * ts:512 * (ts + 1)],
                            start=(cc == 0), stop=(cc == 1))
                if g == 0:  # ScalarE is idle pre-attention: fastest start
                    nc.scalar.copy(dest[g], qp)
                else:
                    nc.vector.tensor_copy(dest[g], qp)

    # ---- Phase 3: attention, head pairs ---------------------------------
    with tc.tile_pool(name="scp", bufs=2, space="PSUM") as scp, \
         tc.tile_pool(name="avp", bufs=1, space="PSUM") as avp, \
         tc.tile_pool(name="expp", bufs=6) as expp, \
         tc.tile_pool(name="nrm", bufs=2) as nrm, \
         tc.tile_pool(name="dsc", bufs=2, space="DRAM") as dscp:
        for pair in range(4):
            g = pair // 2
            hA, hB = 2 * pair, 2 * pair + 1
            last = pair == 3
            av = avp.tile([P, T], F32, tag="av", name="av")
            for s in range(NT):
                exs = []
                for h in (hA, hB):
                    j = h % 4
                    for half in range(2):
                        sc = scp.tile([P, 1024], F32, tag="sc", name="sc")
                        for ts in range(2):
                            tofs = 1024 * half + 512 * ts
                            nc.tensor.matmul(
                                sc[:, 512 * ts:512 * (ts + 1)],
                                lhsT=kT[g][D * j:D * (j + 1), P * s:P * (s + 1)],
                                rhs=qT[g][D * j:D * (j + 1), tofs:tofs + 512],
                                start=True, stop=True,
                                tile_position=(D * j, 0))
                        ex = expp.tile([P, 1024], BF16, tag="ex", name="ex")
                        nc.scalar.activation(out=ex, in_=sc, func=EXP)
                        exs.append((h, half, ex))
                for h, half, ex in exs:
                    col = 0 if h == hA else 64
                    for ts in range(2):
                        tofs = 1024 * half + 512 * ts
                        # A and B share banks at different partition ranges;
                        # per-element has_written makes that safe on HW, but
                        # the sim's bank-granular group tracker would flag it.
                        nc.tensor.matmul(
                            av[col:col + E, tofs:tofs + 512],
                            lhsT=v_sb[:, s, E * h:E * h + E],
                            rhs=ex[:, 512 * ts:512 * (ts + 1)],
                            start=(s == 0), stop=(s == NT - 1),
                            tile_position=(0, col),
                            skip_group_check=True)
            avf = nrm.tile([P, T], F32, tag="avf", name="avf")
            rec = nrm.tile([P, T], F32, tag="rec", name="rec")
            if not last:
                # Evacuate PSUM promptly (two DVE copies; banks free ~5us
                # after the pair's last AV matmul), then off the critical
                # path: reciprocal + DRAM-roundtrip broadcast + multiply.
                for base in (0, 64):
                    nc.vector.tensor_copy(avf[base:base + E, :],
                                          av[base:base + E, :])
                dsc = dscp.tile([2, T], F32, tag="dsc", name="dsc")
                bc = nrm.tile([P, T], F32, tag="bc", name="bc")
                for i, row in enumerate((D, 64 + D)):
                    # reciprocal_approx_fast (custom DVE) gives garbage on HW
                    # via this path; the iterative-divide reciprocal is exact.
                    nc.vector.reciprocal(
                        out=rec[row:row + 1, :], in_=avf[row:row + 1, :])
                    nc.sync.dma_start(out=dsc[i:i + 1, :],
                                      in_=rec[row:row + 1, :])
                for i, base in enumerate((0, 64)):
                    nc.sync.dma_start(out=bc[base:base + D, :],
                                      in_=dsc[i, :].partition_broadcast(D))
                    nc.vector.tensor_mul(
                        nout[pair][base:base + D, :],
                        avf[base:base + D, :],
                        bc[base:base + D, :])
            else:
                # Last pair: no next-pair PSUM pressure.  Shortest chain:
                # DVE reciprocal straight from PSUM while ScalarE (idle now)
                # evacuates the value rows; K=1 PE matmuls broadcast the
                # reciprocal into the idle scores banks.
                rec2 = nrm.tile([P, T], BF16, tag="rec2", name="rec2")
                for i, row in enumerate((D, 64 + D)):
                    nc.vector.reciprocal(
                        out=rec[row:row + 1, :], in_=av[row:row + 1, :])
                    nc.vector.tensor_copy(rec2[row:row + 1, :],
                                          rec[row:row + 1, :])
                for base in (0, 64):
                    nc.scalar.copy(avf[base:base + D, :], av[base:base + D, :])
                for half in range(2):
                    bcp = scp.tile([P, 1024], F32, tag="sc", name="bcp")
                    for colofs, row in ((0, D), (64, 64 + D)):
                        for ts in range(2):
                            tofs = 1024 * half + 512 * ts
                            nc.tensor.matmul(
                                bcp[colofs:colofs + D, 512 * ts:512 * (ts + 1)],
                                lhsT=ones_sb[row:row + 1, :],
                                rhs=rec2[row:row + 1, tofs:tofs + 512],
                                start=True, stop=True,
                                tile_position=((row // 32) * 32, colofs),
                                skip_group_check=True)
                    for base in (0, 64):
                        tofs = 1024 * half
                        nc.vector.tensor_mul(
                            nout[pair][base:base + D, tofs:tofs + 1024],
                            avf[base:base + D, tofs:tofs + 1024],
                            bcp[base:base + D, :])

    # ---- Phase 4: output projection + bias ------------------------------
    with tc.tile_pool(name="prp", bufs=2, space="PSUM") as prp, \
         tc.tile_pool(name="resp", bufs=1) as resp:
        resbig = resp.tile([P, NT, C], F32, name="resbig")
        out_r = out_d.rearrange("(d m p) c -> d p m c", p=P, m=4)
        for n in range(NT):
            rp = prp.tile([P, C], F32, tag="rp", name="rp")
            for q in range(4):
                nc.tensor.matmul(
                    rp,
                    lhsT=nout[q][:, P * n:P * (n + 1)],
                    rhs=wp_sb[:, q, :],
                    start=(q == 0), stop=(q == 3))
            nc.vector.tensor_add(resbig[:, n, :], rp, bias_sb)
            if n % 4 == 3:  # batched output DMA per 4 chunks
                nc.scalar.dma_start(out=out_r[n // 4],
                                    in_=resbig[:, n - 3:n + 1, :])


def declare_io(nc):
    """Declare the kernel's DRAM IO tensors (shared with test harnesses)."""
    x_d = nc.dram_tensor("x", [T, C], BF16, kind="ExternalInput")
    wq_d = nc.dram_tensor("wq", [C, C], BF16, kind="ExternalInput")
    wk_d = nc.dram_tensor("wk", [C, C], BF16, kind="ExternalInput")
    wv_d = nc.dram_tensor("wv", [C, C], BF16, kind="ExternalInput")
    wp_d = nc.dram_tensor("wp", [4, P, C], BF16, kind="ExternalInput")
    bias_d = nc.dram_tensor("bias", [P, C], F32, kind="ExternalInput")
    out_d = nc.dram_tensor("out", [T, C], F32, kind="ExternalOutput")
    return x_d, wq_d, wk_d, wv_d, wp_d, bias_d, out_d


def build_nc():
    nc = bacc.Bacc("TRN2", debug=False, num_devices=N_CORES)
    x_d, wq_d, wk_d, wv_d, wp_d, bias_d, out_d = declare_io(nc)
    with tile.TileContext(nc) as tc:
        with ExitStack() as ctx:
            _body(nc, tc, ctx, x_d.ap(), wq_d.ap(), wk_d.ap(), wv_d.ap(),
                  wp_d.ap(), bias_d.ap(), out_d.ap())
    nc.compile()
    return nc


def prep_inputs(x, wq, wk, wv, wproj, bproj):
    """Host-side reformatting of the full inputs into per-core input maps."""
    f = np.float32
    bf = ml_dtypes.bfloat16
    # [H,C,D] -> [C, H*D]; wq additionally pre-scaled by 1/sqrt(C) (exact).
    wq2 = np.ascontiguousarray(
        np.transpose(np.asarray(wq, f), (1, 0, 2)).reshape(C, H * D)
        * f(1.0 / 16.0)).astype(bf)
    wk2 = np.ascontiguousarray(
        np.transpose(np.asarray(wk, f), (1, 0, 2)).reshape(C, H * D)).astype(bf)
    wv2 = np.ascontiguousarray(
        np.transpose(np.asarray(wv, f), (1, 0, 2)).reshape(C, H * D)).astype(bf)
    # wproj [H*D, C] -> 4 pair-chunks padded to 128 rows:
    # rows 0-31 <- head 2p, rows 64-95 <- head 2p+1, rest zero.
    wp4 = np.zeros((4, P, C), f)
    wproj = np.asarray(wproj, f)
    for p in range(4):
        wp4[p, 0:D] = wproj[64 * p: 64 * p + D]
        wp4[p, 64:64 + D] = wproj[64 * p + D: 64 * p + 2 * D]
    wp4 = wp4.astype(bf)
    bias128 = np.ascontiguousarray(
        np.broadcast_to(np.asarray(bproj, f), (P, C)))
    x_bf = np.asarray(x, f).astype(ml_dtypes.bfloat16)
    in_maps = []
    for b in range(N_CORES):
        in_maps.append({
            "x": np.ascontiguousarray(x_bf[b]),
            "wq": wq2, "wk": wk2, "wv": wv2,
            "wp": wp4, "bias": bias128,
        })
    return in_maps


_NC_CACHE = []


def kernel(x, wq, wk, wv, wproj, bproj, _nc=None):
    in_maps = prep_inputs(x, wq, wk, wv, wproj, bproj)
    if _nc is None:
        if not _NC_CACHE:
            _NC_CACHE.append(build_nc())
        _nc = _NC_CACHE[0]
    res = run_bass_kernel_spmd(_nc, in_maps, list(range(N_CORES)))
    return np.stack([r["out"] for r in res.results], axis=0)


# revision 17
# speedup vs baseline: 8.3062x; 1.0034x over previous
"""Multi-head attention Bass kernel for Trainium2 (8 NeuronCores).

Problem: B=8, T=2048, C=256, H=8, D=32 MHA (dense, full softmax over T).
Sharding: data-parallel over batch -- core b computes batch b end-to-end,
no collectives.  Weights are replicated; per-core x slice is [T, C].

Per-core dataflow (v3):
  1. x is pre-cast to bf16 on the host; xT [C,T] is produced by TWO DMA
     xbar-transpose instructions reading straight from DRAM (no staging,
     no casts, no PE transposes).
  2. qT/kT [D,T] packed 4 heads per [128,T] tile via M=128 matmuls
     (8 MMs per group; group-0 PSUM evacuations on ScalarE so the exp
     stream starts ~8us in); then v [T,D] per head with an appended
     ones-column (v_ext [T,33], bf16).  wq pre-scaled by 1/sqrt(C)=1/16
     host-side.
  3. Scores TRANSPOSED: weiT[s,t] = k[s]*q[t] per head (bf16 matmuls, fp32
     PSUM); ScalarE exp() PSUM->SBUF(bf16): 256 instrs of [128,1024] -- the
     critical path.  Everything else is structured to keep ScalarE fed.
  4. AV: outT_ext[33,t] = v_ext.T @ expw accumulated over s-chunks in PSUM
     (2 heads per tile at partitions 0/64; rows 32/96 are the softmax
     denominators via the ones column).
  5. Normalize with minimal PSUM dwell: two DVE copies evacuate the
     accumulator right after the s-loop (banks free for the next pair
     ~5us after it ends), then off the critical path: reciprocal of the
     denominator rows, broadcast to 32 partitions via a DRAM-roundtrip
     DMA with a zero-stride source AP, and a DVE multiply into nout
     (bf16, zero-padded rows for the projection).  The last pair instead
     broadcasts via K=1 PE matmuls into the idle scores banks (shorter
     chain; no next pair to protect).
  6. Projection: res = sum_pairs noutT_p.T @ wproj_p + bias (bf16 matmuls,
     wproj zero-padded host-side to the pair layout); output DMAs on the
     (idle) ScalarE HWDGE queue.
"""

import numpy as np
import ml_dtypes
from contextlib import ExitStack

import concourse.bass as bass
import concourse.bacc as bacc
import concourse.mybir as mybir
import concourse.tile as tile
from concourse.bass_utils import run_bass_kernel_spmd

B, T, C, H, D = 8, 2048, 256, 8, 32
P = 128
NT = T // P  # 16 chunks of 128 along t / s
F32 = mybir.dt.float32
BF16 = mybir.dt.bfloat16
EXP = mybir.ActivationFunctionType.Exp
N_CORES = 8
E = D + 1  # 33: v columns + ones column


def _body(nc, tc, ctx, x_d, wq_d, wk_d, wv_d, wp_d, bias_d, out_d):
    const = ctx.enter_context(tc.tile_pool(name="const", bufs=1))
    big = ctx.enter_context(tc.tile_pool(name="big", bufs=1))

    wq_sb = const.tile([P, 2, C], BF16)
    wk_sb = const.tile([P, 2, C], BF16)
    wv_sb = const.tile([P, 2, C], BF16)
    wp_sb = const.tile([P, 4, C], BF16)
    bias_sb = const.tile([P, C], F32)
    ones_sb = const.tile([P, D], BF16)
    warm = const.tile([P, 1], F32)

    xT = [big.tile([P, T], BF16, name=f"xT{i}") for i in range(2)]
    qT = [big.tile([P, T], BF16, name=f"qT{i}") for i in range(2)]
    kT = [big.tile([P, T], BF16, name=f"kT{i}") for i in range(2)]
    v_sb = big.tile([P, NT, E * H], BF16)
    nout = [big.tile([P, T], BF16, name=f"nout{i}") for i in range(4)]

    # ---- Phase 1: xT via DRAM xbar transpose FIRST (Tile serializes DMA
    # xbar-mode transitions, so all plain DMAs go after); bf16 weights are
    # DMA'd straight into their SBUF tiles (host pre-casts) ---------------
    for cc, eng in ((0, nc.scalar), (1, nc.sync)):
        eng.dma_start_transpose(
            out=xT[cc], in_=x_d[:, cc * P:(cc + 1) * P])
    for w_sb, w_d, nk in ((wq_sb, wq_d, 2), (wk_sb, wk_d, 2),
                          (wv_sb, wv_d, 2), (wp_sb, wp_d, 4)):
        if nk == 2:
            nc.scalar.dma_start(
                out=w_sb, in_=w_d.rearrange("(k p) c -> p k c", p=P))
        else:
            nc.scalar.dma_start(out=w_sb, in_=w_d.rearrange("q p c -> p q c"))
    nc.scalar.dma_start(out=bias_sb, in_=bias_d)
    nc.gpsimd.memset(v_sb, 1.0)  # ones cols survive; v overwrites the rest
    for t_ in nout:  # rows 32-63 / 96-127 must be 0.0 for the projection
        nc.gpsimd.memset(t_, 0.0)
    nc.vector.memset(ones_sb, 1.0)
    nc.scalar.activation(out=warm, in_=ones_sb[:, 0:1], func=EXP)

    # ---- Phase 2: v (warms the PE), then qT / kT (M=128 matmuls) --------
    with tc.tile_pool(name="pv", bufs=2, space="PSUM") as pv:
        for n in range(NT):
            vp = pv.tile([P, C], F32, tag="vp", name="vp")
            for cc in range(2):
                nc.tensor.matmul(
                    vp,
                    lhsT=xT[cc][:, n * P:(n + 1) * P],
                    rhs=wv_sb[:, cc, :],
                    start=(cc == 0), stop=(cc == 1))
            nc.vector.tensor_copy(
                v_sb[:, n].rearrange("p (h e) -> p h e", e=E)[:, :, 0:D],
                vp.rearrange("p (h d) -> p h d", d=D))
    with tc.tile_pool(name="pq", bufs=2, space="PSUM") as pq:
        for g in range(2):
            for w_sb, dest in ((wq_sb, qT), (wk_sb, kT)):
                qp = pq.tile([P, T], F32, tag="qp", name="qp")
                for ts in range(4):
                    for cc in range(2):
                        nc.tensor.matmul(
                            qp[:, 512 * ts:512 * (ts + 1)],
                            lhsT=w_sb[:, cc, P * g:P * (g + 1)],
                            rhs=xT[cc][:, 512 * ts:512 * (ts + 1)],
                            start=(cc == 0), stop=(cc == 1))
                if g == 0:  # ScalarE is idle pre-attention: fastest start
                    nc.scalar.copy(dest[g], qp)
                else:
                    nc.vector.tensor_copy(dest[g], qp)

    # ---- Phase 3: attention, head pairs ---------------------------------
    with tc.tile_pool(name="scp", bufs=2, space="PSUM") as scp, \
         tc.tile_pool(name="avp", bufs=1, space="PSUM") as avp, \
         tc.tile_pool(name="expp", bufs=8) as expp, \
         tc.tile_pool(name="nrm", bufs=2) as nrm, \
         tc.tile_pool(name="dsc", bufs=2, space="DRAM") as dscp:
        for pair in range(4):
            g = pair // 2
            hA, hB = 2 * pair, 2 * pair + 1
            last = pair == 3
            av = avp.tile([P, T], F32, tag="av", name="av")
            for s in range(NT):
                exs = []
                for h in (hA, hB):
                    j = h % 4
                    for half in range(2):
                        sc = scp.tile([P, 1024], F32, tag="sc", name="sc")
                        for ts in range(2):
                            tofs = 1024 * half + 512 * ts
                            nc.tensor.matmul(
                                sc[:, 512 * ts:512 * (ts + 1)],
                                lhsT=kT[g][D * j:D * (j + 1), P * s:P * (s + 1)],
                                rhs=qT[g][D * j:D * (j + 1), tofs:tofs + 512],
                                start=True, stop=True,
                                tile_position=(D * j, 0))
                        ex = expp.tile([P, 1024], BF16, tag="ex", name="ex")
                        nc.scalar.activation(out=ex, in_=sc, func=EXP)
                        exs.append((h, half, ex))
                for h, half, ex in exs:
                    col = 0 if h == hA else 64
                    for ts in range(2):
                        tofs = 1024 * half + 512 * ts
                        # A and B share banks at different partition ranges;
                        # per-element has_written makes that safe on HW, but
                        # the sim's bank-granular group tracker would flag it.
                        nc.tensor.matmul(
                            av[col:col + E, tofs:tofs + 512],
                            lhsT=v_sb[:, s, E * h:E * h + E],
                            rhs=ex[:, 512 * ts:512 * (ts + 1)],
                            start=(s == 0), stop=(s == NT - 1),
                            tile_position=(0, col),
                            skip_group_check=True)
            avf = nrm.tile([P, T], F32, tag="avf", name="avf")
            rec = nrm.tile([P, T], F32, tag="rec", name="rec")
            if not last:
                # Evacuate PSUM promptly (two DVE copies; banks free ~5us
                # after the pair's last AV matmul), then off the critical
                # path: reciprocal + DRAM-roundtrip broadcast + multiply.
                for base in (0, 64):
                    nc.vector.tensor_copy(avf[base:base + E, :],
                                          av[base:base + E, :])
                dsc = dscp.tile([2, T], F32, tag="dsc", name="dsc")
                bc = nrm.tile([P, T], F32, tag="bc", name="bc")
                for i, row in enumerate((D, 64 + D)):
                    # reciprocal_approx_fast (custom DVE) gives garbage on HW
                    # via this path; the iterative-divide reciprocal is exact.
                    nc.vector.reciprocal(
                        out=rec[row:row + 1, :], in_=avf[row:row + 1, :])
                    nc.sync.dma_start(out=dsc[i:i + 1, :],
                                      in_=rec[row:row + 1, :])
                for i, base in enumerate((0, 64)):
                    nc.sync.dma_start(out=bc[base:base + D, :],
                                      in_=dsc[i, :].partition_broadcast(D))
                    nc.vector.tensor_mul(
                        nout[pair][base:base + D, :],
                        avf[base:base + D, :],
                        bc[base:base + D, :])
            else:
                # Last pair: no next-pair PSUM pressure.  Shortest chain:
                # DVE reciprocal straight from PSUM while ScalarE (idle now)
                # evacuates the value rows; K=1 PE matmuls broadcast the
                # reciprocal into the idle scores banks.
                rec2 = nrm.tile([P, T], BF16, tag="rec2", name="rec2")
                for i, row in enumerate((D, 64 + D)):
                    nc.vector.reciprocal(
                        out=rec[row:row + 1, :], in_=av[row:row + 1, :])
                    nc.vector.tensor_copy(rec2[row:row + 1, :],
                                          rec[row:row + 1, :])
                for base in (0, 64):
                    nc.scalar.copy(avf[base:base + D, :], av[base:base + D, :])
                for half in range(2):
                    bcp = scp.tile([P, 1024], F32, tag="sc", name="bcp")
                    for colofs, row in ((0, D), (64, 64 + D)):
                        for ts in range(2):
                            tofs = 1024 * half + 512 * ts
                            nc.tensor.matmul(
                                bcp[colofs:colofs + D, 512 * ts:512 * (ts + 1)],
                                lhsT=ones_sb[row:row + 1, :],
                                rhs=rec2[row:row + 1, tofs:tofs + 512],
                                start=True, stop=True,
                                tile_position=((row // 32) * 32, colofs),
                                skip_group_check=True)
                    for base in (0, 64):
                        tofs = 1024 * half
                        nc.vector.tensor_mul(
                            nout[pair][base:base + D, tofs:tofs + 1024],
                            avf[base:base + D, tofs:tofs + 1024],
                            bcp[base:base + D, :])

    # ---- Phase 4: output projection + bias ------------------------------
    with tc.tile_pool(name="prp", bufs=2, space="PSUM") as prp, \
         tc.tile_pool(name="resp", bufs=1) as resp:
        resbig = resp.tile([P, NT, C], F32, name="resbig")
        out_r = out_d.rearrange("(n p) c -> n p c", p=P)
        # batched output DMAs, tapering so the last transfer is small
        flush_at = {3: 0, 7: 4, 11: 8, 13: 12, 15: 14}
        for n in range(NT):
            rp = prp.tile([P, C], F32, tag="rp", name="rp")
            for q in range(4):
                nc.tensor.matmul(
                    rp,
                    lhsT=nout[q][:, P * n:P * (n + 1)],
                    rhs=wp_sb[:, q, :],
                    start=(q == 0), stop=(q == 3))
            nc.vector.tensor_add(resbig[:, n, :], rp, bias_sb)
            if n in flush_at:
                lo = flush_at[n]
                nc.scalar.dma_start(
                    out=out_r[lo:n + 1].rearrange("n p c -> p n c"),
                    in_=resbig[:, lo:n + 1, :])


def declare_io(nc):
    """Declare the kernel's DRAM IO tensors (shared with test harnesses)."""
    x_d = nc.dram_tensor("x", [T, C], BF16, kind="ExternalInput")
    wq_d = nc.dram_tensor("wq", [C, C], BF16, kind="ExternalInput")
    wk_d = nc.dram_tensor("wk", [C, C], BF16, kind="ExternalInput")
    wv_d = nc.dram_tensor("wv", [C, C], BF16, kind="ExternalInput")
    wp_d = nc.dram_tensor("wp", [4, P, C], BF16, kind="ExternalInput")
    bias_d = nc.dram_tensor("bias", [P, C], F32, kind="ExternalInput")
    out_d = nc.dram_tensor("out", [T, C], F32, kind="ExternalOutput")
    return x_d, wq_d, wk_d, wv_d, wp_d, bias_d, out_d


def build_nc():
    nc = bacc.Bacc("TRN2", debug=False, num_devices=N_CORES)
    x_d, wq_d, wk_d, wv_d, wp_d, bias_d, out_d = declare_io(nc)
    with tile.TileContext(nc) as tc:
        with ExitStack() as ctx:
            _body(nc, tc, ctx, x_d.ap(), wq_d.ap(), wk_d.ap(), wv_d.ap(),
                  wp_d.ap(), bias_d.ap(), out_d.ap())
    nc.compile()
    return nc


def prep_inputs(x, wq, wk, wv, wproj, bproj):
    """Host-side reformatting of the full inputs into per-core input maps."""
    f = np.float32
    bf = ml_dtypes.bfloat16
    # [H,C,D] -> [C, H*D]; wq additionally pre-scaled by 1/sqrt(C) (exact).
    wq2 = np.ascontiguousarray(
        np.transpose(np.asarray(wq, f), (1, 0, 2)).reshape(C, H * D)
        * f(1.0 / 16.0)).astype(bf)
    wk2 = np.ascontiguousarray(
        np.transpose(np.asarray(wk, f), (1, 0, 2)).reshape(C, H * D)).astype(bf)
    wv2 = np.ascontiguousarray(
        np.transpose(np.asarray(wv, f), (1, 0, 2)).reshape(C, H * D)).astype(bf)
    # wproj [H*D, C] -> 4 pair-chunks padded to 128 rows:
    # rows 0-31 <- head 2p, rows 64-95 <- head 2p+1, rest zero.
    wp4 = np.zeros((4, P, C), f)
    wproj = np.asarray(wproj, f)
    for p in range(4):
        wp4[p, 0:D] = wproj[64 * p: 64 * p + D]
        wp4[p, 64:64 + D] = wproj[64 * p + D: 64 * p + 2 * D]
    wp4 = wp4.astype(bf)
    bias128 = np.ascontiguousarray(
        np.broadcast_to(np.asarray(bproj, f), (P, C)))
    x_bf = np.asarray(x, f).astype(ml_dtypes.bfloat16)
    in_maps = []
    for b in range(N_CORES):
        in_maps.append({
            "x": np.ascontiguousarray(x_bf[b]),
            "wq": wq2, "wk": wk2, "wv": wv2,
            "wp": wp4, "bias": bias128,
        })
    return in_maps


_NC_CACHE = []


def kernel(x, wq, wk, wv, wproj, bproj, _nc=None):
    in_maps = prep_inputs(x, wq, wk, wv, wproj, bproj)
    if _nc is None:
        if not _NC_CACHE:
            _NC_CACHE.append(build_nc())
        _nc = _NC_CACHE[0]
    res = run_bass_kernel_spmd(_nc, in_maps, list(range(N_CORES)))
    return np.stack([r["out"] for r in res.results], axis=0)
